# revision 40
# baseline (speedup 1.0000x reference)
"""Trainium2 Bass kernel for AttnSum3d pooling.

Math (per batch):
    xm = input * mask[:, None]                      # [L, D]
    S  = xm @ xm.T                                  # [L, L], symmetric
    w  = softmax(S, axis=0)  (columns sum to 1 over rows)
    out[d]       = (1/L) * sum_m sum_l w[l, m] xm[l, d]
    attn_mean[m] = (1/L) * sum_l w[l, m] = 1/L      (constant!)

Per m-block of 128 columns (stored row-wise thanks to symmetry):
    c[m]      >= max_l S[l, m]  via  sqrt(nsq[m]*max nsq) (Cauchy-Schwarz),
                 computed with a DVE exponent-halving sqrt (x1.06 margin)
    P_j[m, l] = exp(S_j[m, l] - c[m])     (ACT, bias=-c, accum_out=colsum)
    v[m]      = 1 / (L * colsum[m])
    r[l]     += sum_m v[m] * P_j[m, l]    (TensorE, lhsT=v, bf16)
    out[d]    = sum_l r[l] * xm[l, d]     (DVE accumulation + one
                                           partition-sum matmul)

Data-parallel over batch: 16 batches -> 8 cores x 2 batches. Batch 1's
prep and batch 0's tail are interleaved into the main loops to keep the
ACT engine (the bottleneck) fed continuously.
"""

import sys

for _p in ("/opt/trn_rl_repo",):
    if _p not in sys.path:
        sys.path.insert(0, _p)

import numpy as np

B, L, D = 16, 2048, 128
NCORES = 8
BPC = B // NCORES          # batches per core
NT = L // 128              # 16 tiles of 128 along L
TPB = L // 128             # L-rows per partition in the (p t) layout

_CACHE = {}


def _build_nc(batches=BPC):
    import concourse.bacc as bacc
    import concourse.tile as tile
    from concourse import mybir
    from concourse.masks import make_identity

    f32 = mybir.dt.float32
    bf16 = mybir.dt.bfloat16
    AF = mybir.ActivationFunctionType
    ALU = mybir.AluOpType
    AX = mybir.AxisListType

    nc = bacc.Bacc("TRN2", target_bir_lowering=False, debug=False)

    x_d = nc.dram_tensor("input", [BPC, L, D], f32, kind="ExternalInput").ap()
    m_d = nc.dram_tensor("mask", [BPC, L], f32, kind="ExternalInput").ap()
    o_d = nc.dram_tensor("out", [BPC, D], f32, kind="ExternalOutput").ap()

    with tile.TileContext(nc) as tc:
        with (
            tc.tile_pool(name="consts", bufs=1) as consts,
            tc.tile_pool(name="xb", bufs=2) as xb,
            tc.tile_pool(name="pb", bufs=8) as pb,
            tc.tile_pool(name="small", bufs=8) as small,
            tc.tile_pool(name="acc", bufs=2) as accp,
            tc.tile_pool(name="psS", bufs=2, space="PSUM") as psS,
            tc.tile_pool(name="psR", bufs=1, space="PSUM") as psR,
        ):
            identity = consts.tile([128, 128], f32)
            make_identity(nc, identity)
            ones_row = consts.tile([1, 128], f32)
            nc.vector.memset(ones_row, 1.0)
            ones_col = consts.tile([128, 1], f32)
            nc.vector.memset(ones_col, 1.0)

            ctxs = [{} for _ in range(batches)]
            # deferred closures, dependency-ordered and hand-interleaved so
            # PE-heavy ops (transposes) spread ~1-2 per block and never
            # crowd out S-matmul production in the static PE schedule
            deferred = []

            def drain(k):
                for _ in range(k):
                    if deferred:
                        deferred.pop(0)()

            def emit_loads(b):
                c = ctxs[b]
                c["xpa"] = xb.tile([128, 8, D], f32, name=f"xpa{b}", tag="xpa")
                c["xpb"] = xb.tile([128, 8, D], f32, name=f"xpb{b}", tag="xpb")
                xsrc = x_d[b].rearrange("(p t) d -> p t d", p=128)
                nc.sync.dma_start(out=c["xpa"][:, 0:4, :], in_=xsrc[:, 0:4, :])
                nc.sync.dma_start(out=c["xpa"][:, 4:8, :], in_=xsrc[:, 4:8, :])
                nc.sync.dma_start(out=c["xpb"][:, 0:4, :], in_=xsrc[:, 8:12, :])
                nc.sync.dma_start(out=c["xpb"][:, 4:8, :], in_=xsrc[:, 12:16, :])
                c["mask"] = xb.tile([128, TPB], f32, name=f"mask{b}", tag="mask")
                nc.sync.dma_start(
                    out=c["mask"][:], in_=m_d[b].rearrange("(p t) -> p t", p=128)
                )

            def xp_t(c, t):
                return c["xpa"][:, t, :] if t < 8 else c["xpb"][:, t - 8, :]

            def prep_ops(b):
                """Dependency-ordered closures for mask/nsq/negc/xmT with
                PE transposes interleaved between DVE ops."""
                c = ctxs[b]
                c["xm"] = xb.tile([128, NT, D], f32, name=f"xm{b}", tag="xm")
                c["nsq"] = xb.tile([128, NT], f32, name=f"nsq{b}", tag="nsq")
                c["xmT"] = xb.tile([128, L], bf16, name=f"xmT{b}", tag="xmT")
                masks = []
                nsqs = []
                trs = []
                finals = []

                eng = nc.vector if b == 0 else nc.gpsimd
                for t in range(NT):
                    def _mask(t=t):
                        eng.tensor_scalar_mul(
                            c["xm"][:, t, :], xp_t(c, t), c["mask"][:, t : t + 1]
                        )
                    masks.append(_mask)

                # nsq: batch 0 on the idle ACT (also pre-warms the exp
                # table); later batches on DVE (ACT is busy by then)
                for t in range(NT):
                    if b == 0:
                        def _nsq(t=t):
                            sqj = small.tile(
                                [128, D], f32, name=f"sqj{b}_{t}", tag="sqj"
                            )
                            nc.scalar.activation(
                                sqj[:],
                                xp_t(c, t),
                                AF.Square,
                                scale=c["mask"][:, t : t + 1],
                                accum_out=c["nsq"][:, t : t + 1],
                            )
                        nsqs.append(_nsq)
                    else:
                        def _nsq(t=t):
                            sqj = small.tile(
                                [128, D], f32, name=f"sqj{b}_{t}", tag="sqj"
                            )
                            nc.gpsimd.tensor_mul(
                                sqj[:], c["xm"][:, t, :], c["xm"][:, t, :]
                            )
                            nc.vector.reduce_sum(
                                c["nsq"][:, t : t + 1], sqj[:], AX.X
                            )
                        nsqs.append(_nsq)

                def _n2max_a():
                    c["nmaxp"] = xb.tile([128, 1], f32, name=f"nmx{b}", tag="nmx")
                    nc.vector.reduce_max(c["nmaxp"][:], c["nsq"][:], AX.X)
                    tps = psS.tile([1, 128], f32, name=f"tpn{b}", tag="ps")
                    nc.tensor.transpose(tps[:], c["nmaxp"][:], identity[:])
                    c["nmr"] = xb.tile([1, 128], f32, name=f"nmr{b}", tag="nmr")
                    nc.vector.tensor_copy(c["nmr"][:], tps[:])
                finals.append(_n2max_a)

                def _n2max_b():
                    n2max = xb.tile([1, 1], f32, name=f"n2m{b}", tag="n2m")
                    nc.vector.reduce_max(n2max[:], c["nmr"][:], AX.X)
                    bps = psS.tile([128, 1], f32, name=f"bps{b}", tag="ps")
                    nc.tensor.matmul(
                        bps[:], ones_row[:], n2max[:], start=True, stop=True
                    )
                    c["n2b"] = xb.tile([128, 1], f32, name=f"n2b{b}", tag="n2b")
                    nc.vector.tensor_copy(c["n2b"][:], bps[:])
                finals.append(_n2max_b)

                def _negc():
                    zt = xb.tile([128, NT], f32, name=f"zt{b}", tag="zt")
                    nc.vector.tensor_scalar_mul(zt[:], c["nsq"][:], c["n2b"][:, 0:1])
                    zi = zt[:].bitcast(mybir.dt.int32)
                    nc.vector.tensor_scalar(zi, zi, 1, None, op0=ALU.arith_shift_right)
                    nc.vector.tensor_scalar(zi, zi, 0x1FC00000, None, op0=ALU.add)
                    c["negc"] = xb.tile([128, NT], f32, name=f"negc{b}", tag="negc")
                    nc.vector.tensor_scalar_mul(c["negc"][:], zt[:], -1.06)
                finals.append(_negc)

                for t in range(NT):
                    def _tr(t=t):
                        tp = psS.tile([128, 128], f32, name=f"tp{b}_{t}", tag="ps")
                        nc.tensor.transpose(tp[:], c["xm"][:, t, :], identity[:])
                        nc.vector.tensor_copy(
                            c["xmT"][:, t * 128 : (t + 1) * 128], tp[:]
                        )
                    trs.append(_tr)
                # interleave: all masks, then alternate nsq/transpose
                ops = list(masks)
                for t in range(NT):
                    ops.append(nsqs[t])
                    ops.append(trs[t])
                ops.extend(finals)
                return ops

            def emit_main(b):
                c = ctxs[b]
                c["r_ps"] = psR.tile([1, L], f32, name=f"r_ps{b}", tag="r")
                xmT = c["xmT"]
                pend = {}  # jb -> (vjb, Ph): vP runs one block behind so the
                # PE never waits on the exp/v-chain of the current block

                def emit_vP(jb):
                    vjb, Ph = pend.pop(jb)
                    for h in range(2):
                        for k in range(2):
                            nc.tensor.matmul(
                                c["r_ps"][
                                    0:1, h * 1024 + k * 512 : h * 1024 + (k + 1) * 512
                                ],
                                vjb[:],
                                Ph[h][:, k * 512 : (k + 1) * 512],
                                start=(jb == 0),
                                stop=(jb == NT - 1),
                            )

                for jb in range(NT):
                    lhsT = xmT[:, jb * 128 : (jb + 1) * 128]
                    csum = small.tile([128, 2], f32, name=f"cs{b}_{jb}", tag="cs")
                    Ph = []
                    for h in range(2):
                        S_ps = psS.tile(
                            [128, 1024], f32, name=f"S{b}_{jb}_{h}", tag="ps"
                        )
                        for k in range(2):
                            nc.tensor.matmul(
                                S_ps[:, k * 512 : (k + 1) * 512],
                                lhsT,
                                xmT[
                                    :, h * 1024 + k * 512 : h * 1024 + (k + 1) * 512
                                ],
                                start=True,
                                stop=True,
                            )
                        P = pb.tile([128, 1024], bf16, name=f"P{b}_{jb}_{h}", tag="P")
                        nc.scalar.activation(
                            P[:],
                            S_ps[:],
                            AF.Exp,
                            bias=c["negc"][:, jb : jb + 1],
                            scale=1.0,
                            accum_out=csum[:, h : h + 1],
                        )
                        Ph.append(P)

                    cst = small.tile([128, 1], f32, name=f"cst{b}_{jb}", tag="cst")
                    nc.vector.tensor_add(cst[:], csum[:, 0:1], csum[:, 1:2])
                    vj = small.tile([128, 1], f32, name=f"vj{b}_{jb}", tag="vj")
                    nc.vector.reciprocal(vj[:], cst[:])
                    vjb = small.tile([128, 1], bf16, name=f"vjb{b}_{jb}", tag="vjb")
                    nc.vector.tensor_scalar_mul(vjb[:], vj[:], 1.0 / L)
                    pend[jb] = (vjb, Ph)
                    if jb > 0:
                        emit_vP(jb - 1)
                    drain(4)
                emit_vP(NT - 1)

                c["r_sb"] = xb.tile([1, L], f32, name=f"r_sb{b}", tag="r_sb")

            def tail_ops(b):
                """r-psum drain + rT transposes + out accumulation, in
                dependency order with PE ops spread out."""
                c = ctxs[b]
                c["rT"] = xb.tile([128, NT], f32, name=f"rT{b}", tag="rT")
                cps = []
                for q in range(4):
                    def _cp(q=q):
                        nc.vector.tensor_copy(
                            c["r_sb"][0:1, q * 512 : (q + 1) * 512],
                            c["r_ps"][0:1, q * 512 : (q + 1) * 512],
                        )
                    cps.append(_cp)
                rts = []
                accs = []
                for i in range(NT):
                    def _rt(i=i):
                        tpr = psS.tile([128, 1], f32, name=f"tpr{b}_{i}", tag="ps")
                        nc.tensor.transpose(
                            tpr[:],
                            c["r_sb"][0:1, i * 128 : (i + 1) * 128],
                            identity[0:1, 0:1],
                        )
                        nc.vector.tensor_copy(c["rT"][:, i : i + 1], tpr[:])
                    rts.append(_rt)

                def _acc0():
                    a = accp.tile([128, D], f32, name=f"acc{b}_0", tag=f"acc{b}")
                    nc.vector.tensor_scalar_mul(
                        a[:], c["xm"][:, 0, :], c["rT"][:, 0:1]
                    )
                    c["acc"] = a
                accs.append(_acc0)
                for i in range(1, NT):
                    def _acci(i=i):
                        a = accp.tile([128, D], f32, name=f"acc{b}_{i}", tag=f"acc{b}")
                        nc.vector.scalar_tensor_tensor(
                            out=a[:],
                            in0=c["xm"][:, i, :],
                            scalar=c["rT"][:, i : i + 1],
                            in1=c["acc"][:],
                            op0=ALU.mult,
                            op1=ALU.add,
                        )
                        c["acc"] = a
                    accs.append(_acci)

                def _fin():
                    o_ps = psS.tile([1, D], f32, name=f"o_ps{b}", tag="ps")
                    nc.tensor.matmul(
                        o_ps[:], ones_col[:], c["acc"][:], start=True, stop=True
                    )
                    o_sb = xb.tile([1, D], f32, name=f"o_sb{b}", tag="o_sb")
                    nc.vector.tensor_copy(o_sb[:], o_ps[:])
                    nc.sync.dma_start(out=o_d[b : b + 1, :], in_=o_sb[:])
                ops = []
                for i in range(NT):
                    if i % 4 == 0:
                        ops.append(cps[i // 4])
                    ops.append(rts[i])
                    ops.append(accs[i])
                ops.append(_fin)
                return ops

            # ---------------- emission schedule ----------------
            for b in range(batches):
                emit_loads(b)
            for op in prep_ops(0):
                op()
            for b in range(batches):
                if b + 1 < batches:
                    deferred.extend(prep_ops(b + 1))
                emit_main(b)  # drains deferred (prep of b+1 / tail of b-1)
                deferred.extend(tail_ops(b))
            while deferred:
                deferred.pop(0)()

    nc.compile()
    return nc


def _get_nc():
    import os

    batches = int(os.environ.get("K_BATCHES", str(BPC)))
    key = ("nc", batches)
    if key not in _CACHE:
        _CACHE[key] = _build_nc(batches=batches)
    return _CACHE[key]


def _enable_tracing():
    """Shim antenv.axon_hooks (absent in this container) so
    run_bass_kernel_spmd(trace=True) can capture NTFF profiles through
    the axon .so, and neutralize the S3 artifact upload."""
    if _CACHE.get("trace_shim"):
        return
    import types

    import antenv

    if not hasattr(antenv, "axon_hooks"):
        mod = types.ModuleType("antenv.axon_hooks")
        mod._hook = None

        def set_axon_ntff_profile_hook(h):
            mod._hook = h

        def get_axon_ntff_profile_hook():
            return mod._hook

        mod.set_axon_ntff_profile_hook = set_axon_ntff_profile_hook
        mod.get_axon_ntff_profile_hook = get_axon_ntff_profile_hook
        sys.modules["antenv.axon_hooks"] = mod
        antenv.axon_hooks = mod

    from antenv.axon_hooks import get_axon_ntff_profile_hook, set_axon_ntff_profile_hook

    if get_axon_ntff_profile_hook() is None:
        if "/root/.axon_site" not in sys.path:
            sys.path.insert(0, "/root/.axon_site")
        from trn_agent_boot.trn_boot import _ntff_profile_via_ctypes

        set_axon_ntff_profile_hook(
            _ntff_profile_via_ctypes("/opt/axon/libaxon_pjrt.so")
        )

    import concourse.bass_utils as bu

    bu.upload_artifacts = lambda tmpdir: f"local://{tmpdir}"
    _CACHE["trace_shim"] = True


def _in_maps(inputs):
    x = np.ascontiguousarray(np.asarray(inputs["input"], dtype=np.float32))
    m = np.ascontiguousarray(np.asarray(inputs["mask"], dtype=np.float32))
    assert x.shape == (B, L, D) and m.shape == (B, L)
    return [
        {
            "input": np.ascontiguousarray(x[c * BPC : (c + 1) * BPC]),
            "mask": np.ascontiguousarray(m[c * BPC : (c + 1) * BPC]),
        }
        for c in range(NCORES)
    ]


def _run(inputs, trace=False, **kw):
    from concourse.bass_utils import run_bass_kernel_spmd

    if trace:
        _enable_tracing()
    nc = _get_nc()
    res = run_bass_kernel_spmd(
        nc, _in_maps(inputs), core_ids=list(range(NCORES)), trace=trace, **kw
    )
    outs = np.stack([res.results[c]["out"] for c in range(NCORES)])  # [8, BPC, D]
    out_full = outs.reshape(B, 1, D).astype(np.float32)
    attn_mean = np.full((B, L), 1.0 / L, dtype=np.float32)
    return (out_full, attn_mean), res


def kernel(**inputs):
    (out_full, attn_mean), _ = _run(inputs, trace=False)
    return (out_full, attn_mean)


# revision 41
# speedup vs baseline: 1.0256x; 1.0256x over previous
"""Trainium2 Bass kernel for AttnSum3d pooling.

Math (per batch):
    xm = input * mask[:, None]                      # [L, D]
    S  = xm @ xm.T                                  # [L, L], symmetric
    w  = softmax(S, axis=0)  (columns sum to 1 over rows)
    out[d]       = (1/L) * sum_m sum_l w[l, m] xm[l, d]
    attn_mean[m] = (1/L) * sum_l w[l, m] = 1/L      (constant!)

Per m-block of 128 columns (stored row-wise thanks to symmetry):
    c[m]      >= max_l S[l, m]  via  sqrt(nsq[m]*max nsq) (Cauchy-Schwarz),
                 computed with a DVE exponent-halving sqrt (x1.06 margin)
    P_j[m, l] = exp(S_j[m, l] - c[m])     (ACT, bias=-c, accum_out=colsum)
    v[m]      = 1 / (L * colsum[m])
    r[l]     += sum_m v[m] * P_j[m, l]    (TensorE, lhsT=v, bf16)
    out[d]    = sum_l r[l] * xm[l, d]     (DVE accumulation + one
                                           partition-sum matmul)

Data-parallel over batch: 16 batches -> 8 cores x 2 batches. Batch 1's
prep and batch 0's tail are interleaved into the main loops to keep the
ACT engine (the bottleneck) fed continuously.
"""

import sys

for _p in ("/opt/trn_rl_repo",):
    if _p not in sys.path:
        sys.path.insert(0, _p)

import numpy as np

B, L, D = 16, 2048, 128
NCORES = 8
BPC = B // NCORES          # batches per core
NT = L // 128              # 16 tiles of 128 along L
TPB = L // 128             # L-rows per partition in the (p t) layout

_CACHE = {}


def _build_nc(batches=BPC):
    import concourse.bacc as bacc
    import concourse.tile as tile
    from concourse import mybir
    from concourse.masks import make_identity

    f32 = mybir.dt.float32
    bf16 = mybir.dt.bfloat16
    AF = mybir.ActivationFunctionType
    ALU = mybir.AluOpType
    AX = mybir.AxisListType

    nc = bacc.Bacc("TRN2", target_bir_lowering=False, debug=False)

    x_d = nc.dram_tensor("input", [BPC, L, D], f32, kind="ExternalInput").ap()
    m_d = nc.dram_tensor("mask", [BPC, L], f32, kind="ExternalInput").ap()
    o_d = nc.dram_tensor("out", [BPC, D], f32, kind="ExternalOutput").ap()

    with tile.TileContext(nc) as tc:
        with (
            tc.tile_pool(name="consts", bufs=1) as consts,
            tc.tile_pool(name="xb", bufs=2) as xb,
            tc.tile_pool(name="pb", bufs=8) as pb,
            tc.tile_pool(name="small", bufs=8) as small,
            tc.tile_pool(name="acc", bufs=2) as accp,
            tc.tile_pool(name="psS", bufs=2, space="PSUM") as psS,
            tc.tile_pool(name="psR", bufs=1, space="PSUM") as psR,
        ):
            identity = consts.tile([128, 128], f32)
            make_identity(nc, identity)
            ones_row = consts.tile([1, 128], f32)
            nc.vector.memset(ones_row, 1.0)
            ones_col = consts.tile([128, 1], f32)
            nc.vector.memset(ones_col, 1.0)

            ctxs = [{} for _ in range(batches)]
            # deferred closures, dependency-ordered and hand-interleaved so
            # PE-heavy ops (transposes) spread ~1-2 per block and never
            # crowd out S-matmul production in the static PE schedule
            deferred = []

            def drain(k):
                for _ in range(k):
                    if deferred:
                        deferred.pop(0)()

            def emit_loads(b):
                c = ctxs[b]
                c["xpa"] = xb.tile([128, 8, D], f32, name=f"xpa{b}", tag="xpa")
                c["xpb"] = xb.tile([128, 8, D], f32, name=f"xpb{b}", tag="xpb")
                xsrc = x_d[b].rearrange("(p t) d -> p t d", p=128)
                nc.sync.dma_start(out=c["xpa"][:, 0:4, :], in_=xsrc[:, 0:4, :])
                nc.sync.dma_start(out=c["xpa"][:, 4:8, :], in_=xsrc[:, 4:8, :])
                nc.sync.dma_start(out=c["xpb"][:, 0:4, :], in_=xsrc[:, 8:12, :])
                nc.sync.dma_start(out=c["xpb"][:, 4:8, :], in_=xsrc[:, 12:16, :])
                c["mask"] = xb.tile([128, TPB], f32, name=f"mask{b}", tag="mask")
                nc.sync.dma_start(
                    out=c["mask"][:], in_=m_d[b].rearrange("(p t) -> p t", p=128)
                )

            def xp_t(c, t):
                return c["xpa"][:, t, :] if t < 8 else c["xpb"][:, t - 8, :]

            def prep_ops(b):
                """Dependency-ordered closures for mask/nsq/negc/xmT with
                PE transposes interleaved between DVE ops."""
                c = ctxs[b]
                c["xm"] = xb.tile([128, NT, D], f32, name=f"xm{b}", tag="xm")
                c["nsq"] = xb.tile([128, NT], f32, name=f"nsq{b}", tag="nsq")
                c["xmT"] = xb.tile([128, L], bf16, name=f"xmT{b}", tag="xmT")
                masks = []
                nsqs = []
                trs = []
                finals = []

                for t in range(NT):
                    def _mask(t=t):
                        nc.vector.tensor_scalar_mul(
                            c["xm"][:, t, :], xp_t(c, t), c["mask"][:, t : t + 1]
                        )
                    masks.append(_mask)

                # nsq: batch 0 on the idle ACT (also pre-warms the exp
                # table); later batches on DVE (ACT is busy by then)
                for t in range(NT):
                    if b == 0:
                        def _nsq(t=t):
                            sqj = small.tile(
                                [128, D], f32, name=f"sqj{b}_{t}", tag="sqj"
                            )
                            nc.scalar.activation(
                                sqj[:],
                                xp_t(c, t),
                                AF.Square,
                                scale=c["mask"][:, t : t + 1],
                                accum_out=c["nsq"][:, t : t + 1],
                            )
                        nsqs.append(_nsq)
                    else:
                        def _nsq(t=t):
                            sqj = small.tile(
                                [128, D], f32, name=f"sqj{b}_{t}", tag="sqj"
                            )
                            nc.vector.tensor_mul(
                                sqj[:], c["xm"][:, t, :], c["xm"][:, t, :]
                            )
                            nc.vector.reduce_sum(
                                c["nsq"][:, t : t + 1], sqj[:], AX.X
                            )
                        nsqs.append(_nsq)

                def _n2max_a():
                    c["nmaxp"] = xb.tile([128, 1], f32, name=f"nmx{b}", tag="nmx")
                    nc.vector.reduce_max(c["nmaxp"][:], c["nsq"][:], AX.X)
                    tps = psS.tile([1, 128], f32, name=f"tpn{b}", tag="ps")
                    nc.tensor.transpose(tps[:], c["nmaxp"][:], identity[:])
                    c["nmr"] = xb.tile([1, 128], f32, name=f"nmr{b}", tag="nmr")
                    nc.vector.tensor_copy(c["nmr"][:], tps[:])
                finals.append(_n2max_a)

                def _n2max_b():
                    n2max = xb.tile([1, 1], f32, name=f"n2m{b}", tag="n2m")
                    nc.vector.reduce_max(n2max[:], c["nmr"][:], AX.X)
                    bps = psS.tile([128, 1], f32, name=f"bps{b}", tag="ps")
                    nc.tensor.matmul(
                        bps[:], ones_row[:], n2max[:], start=True, stop=True
                    )
                    c["n2b"] = xb.tile([128, 1], f32, name=f"n2b{b}", tag="n2b")
                    nc.vector.tensor_copy(c["n2b"][:], bps[:])
                finals.append(_n2max_b)

                def _negc():
                    zt = xb.tile([128, NT], f32, name=f"zt{b}", tag="zt")
                    nc.vector.tensor_scalar_mul(zt[:], c["nsq"][:], c["n2b"][:, 0:1])
                    zi = zt[:].bitcast(mybir.dt.int32)
                    nc.vector.tensor_scalar(zi, zi, 1, None, op0=ALU.arith_shift_right)
                    nc.vector.tensor_scalar(zi, zi, 0x1FC00000, None, op0=ALU.add)
                    c["negc"] = xb.tile([128, NT], f32, name=f"negc{b}", tag="negc")
                    nc.vector.tensor_scalar_mul(c["negc"][:], zt[:], -1.06)
                finals.append(_negc)

                for t in range(NT):
                    def _tr(t=t):
                        tp = psS.tile([128, 128], f32, name=f"tp{b}_{t}", tag="ps")
                        nc.tensor.transpose(tp[:], c["xm"][:, t, :], identity[:])
                        nc.vector.tensor_copy(
                            c["xmT"][:, t * 128 : (t + 1) * 128], tp[:]
                        )
                    trs.append(_tr)
                # interleave: all masks, then alternate nsq/transpose
                ops = list(masks)
                for t in range(NT):
                    ops.append(nsqs[t])
                    ops.append(trs[t])
                ops.extend(finals)
                return ops

            def emit_main(b):
                c = ctxs[b]
                c["r_ps"] = psR.tile([1, L], f32, name=f"r_ps{b}", tag="r")
                xmT = c["xmT"]
                pend = {}  # jb -> (vjb, Ph): vP runs one block behind so the
                # PE never waits on the exp/v-chain of the current block

                def emit_vP(jb):
                    vjb, Ph = pend.pop(jb)
                    for h in range(2):
                        for k in range(2):
                            nc.tensor.matmul(
                                c["r_ps"][
                                    0:1, h * 1024 + k * 512 : h * 1024 + (k + 1) * 512
                                ],
                                vjb[:],
                                Ph[h][:, k * 512 : (k + 1) * 512],
                                start=(jb == 0),
                                stop=(jb == NT - 1),
                            )

                for jb in range(NT):
                    lhsT = xmT[:, jb * 128 : (jb + 1) * 128]
                    csum = small.tile([128, 2], f32, name=f"cs{b}_{jb}", tag="cs")
                    Ph = []
                    for h in range(2):
                        S_ps = psS.tile(
                            [128, 1024], f32, name=f"S{b}_{jb}_{h}", tag="ps"
                        )
                        for k in range(2):
                            nc.tensor.matmul(
                                S_ps[:, k * 512 : (k + 1) * 512],
                                lhsT,
                                xmT[
                                    :, h * 1024 + k * 512 : h * 1024 + (k + 1) * 512
                                ],
                                start=True,
                                stop=True,
                            )
                        P = pb.tile([128, 1024], bf16, name=f"P{b}_{jb}_{h}", tag="P")
                        nc.scalar.activation(
                            P[:],
                            S_ps[:],
                            AF.Exp,
                            bias=c["negc"][:, jb : jb + 1],
                            scale=1.0,
                            accum_out=csum[:, h : h + 1],
                        )
                        Ph.append(P)

                    cst = small.tile([128, 1], f32, name=f"cst{b}_{jb}", tag="cst")
                    nc.vector.tensor_add(cst[:], csum[:, 0:1], csum[:, 1:2])
                    vj = small.tile([128, 1], f32, name=f"vj{b}_{jb}", tag="vj")
                    nc.vector.reciprocal(vj[:], cst[:])
                    vjb = small.tile([128, 1], bf16, name=f"vjb{b}_{jb}", tag="vjb")
                    nc.vector.tensor_scalar_mul(vjb[:], vj[:], 1.0 / L)
                    pend[jb] = (vjb, Ph)
                    if jb > 0:
                        emit_vP(jb - 1)
                    drain(4)
                emit_vP(NT - 1)

                c["r_sb"] = xb.tile([1, L], f32, name=f"r_sb{b}", tag="r_sb")

            def tail_ops(b):
                """r-psum drain + rT transposes + out accumulation, in
                dependency order with PE ops spread out."""
                c = ctxs[b]
                c["rT"] = xb.tile([128, NT], f32, name=f"rT{b}", tag="rT")
                cps = []
                for q in range(4):
                    def _cp(q=q):
                        nc.vector.tensor_copy(
                            c["r_sb"][0:1, q * 512 : (q + 1) * 512],
                            c["r_ps"][0:1, q * 512 : (q + 1) * 512],
                        )
                    cps.append(_cp)
                rts = []
                accs = []
                for i in range(NT):
                    def _rt(i=i):
                        tpr = psS.tile([128, 1], f32, name=f"tpr{b}_{i}", tag="ps")
                        nc.tensor.transpose(
                            tpr[:],
                            c["r_sb"][0:1, i * 128 : (i + 1) * 128],
                            identity[0:1, 0:1],
                        )
                        nc.vector.tensor_copy(c["rT"][:, i : i + 1], tpr[:])
                    rts.append(_rt)

                def _acc0():
                    a = accp.tile([128, D], f32, name=f"acc{b}_0", tag=f"acc{b}")
                    nc.vector.tensor_scalar_mul(
                        a[:], c["xm"][:, 0, :], c["rT"][:, 0:1]
                    )
                    c["acc"] = a
                accs.append(_acc0)
                for i in range(1, NT):
                    def _acci(i=i):
                        a = accp.tile([128, D], f32, name=f"acc{b}_{i}", tag=f"acc{b}")
                        nc.vector.scalar_tensor_tensor(
                            out=a[:],
                            in0=c["xm"][:, i, :],
                            scalar=c["rT"][:, i : i + 1],
                            in1=c["acc"][:],
                            op0=ALU.mult,
                            op1=ALU.add,
                        )
                        c["acc"] = a
                    accs.append(_acci)

                def _fin():
                    o_ps = psS.tile([1, D], f32, name=f"o_ps{b}", tag="ps")
                    nc.tensor.matmul(
                        o_ps[:], ones_col[:], c["acc"][:], start=True, stop=True
                    )
                    o_sb = xb.tile([1, D], f32, name=f"o_sb{b}", tag="o_sb")
                    nc.vector.tensor_copy(o_sb[:], o_ps[:])
                    nc.sync.dma_start(out=o_d[b : b + 1, :], in_=o_sb[:])
                ops = []
                for i in range(NT):
                    if i % 4 == 0:
                        ops.append(cps[i // 4])
                    ops.append(rts[i])
                    ops.append(accs[i])
                ops.append(_fin)
                return ops

            # ---------------- emission schedule ----------------
            for b in range(batches):
                emit_loads(b)
            for op in prep_ops(0):
                op()
            for b in range(batches):
                if b + 1 < batches:
                    deferred.extend(prep_ops(b + 1))
                emit_main(b)  # drains deferred (prep of b+1 / tail of b-1)
                deferred.extend(tail_ops(b))
            while deferred:
                deferred.pop(0)()

    nc.compile()
    return nc


def _get_nc():
    import os

    batches = int(os.environ.get("K_BATCHES", str(BPC)))
    key = ("nc", batches)
    if key not in _CACHE:
        _CACHE[key] = _build_nc(batches=batches)
    return _CACHE[key]


def _enable_tracing():
    """Shim antenv.axon_hooks (absent in this container) so
    run_bass_kernel_spmd(trace=True) can capture NTFF profiles through
    the axon .so, and neutralize the S3 artifact upload."""
    if _CACHE.get("trace_shim"):
        return
    import types

    import antenv

    if not hasattr(antenv, "axon_hooks"):
        mod = types.ModuleType("antenv.axon_hooks")
        mod._hook = None

        def set_axon_ntff_profile_hook(h):
            mod._hook = h

        def get_axon_ntff_profile_hook():
            return mod._hook

        mod.set_axon_ntff_profile_hook = set_axon_ntff_profile_hook
        mod.get_axon_ntff_profile_hook = get_axon_ntff_profile_hook
        sys.modules["antenv.axon_hooks"] = mod
        antenv.axon_hooks = mod

    from antenv.axon_hooks import get_axon_ntff_profile_hook, set_axon_ntff_profile_hook

    if get_axon_ntff_profile_hook() is None:
        if "/root/.axon_site" not in sys.path:
            sys.path.insert(0, "/root/.axon_site")
        from trn_agent_boot.trn_boot import _ntff_profile_via_ctypes

        set_axon_ntff_profile_hook(
            _ntff_profile_via_ctypes("/opt/axon/libaxon_pjrt.so")
        )

    import concourse.bass_utils as bu

    bu.upload_artifacts = lambda tmpdir: f"local://{tmpdir}"
    _CACHE["trace_shim"] = True


def _in_maps(inputs):
    x = np.ascontiguousarray(np.asarray(inputs["input"], dtype=np.float32))
    m = np.ascontiguousarray(np.asarray(inputs["mask"], dtype=np.float32))
    assert x.shape == (B, L, D) and m.shape == (B, L)
    return [
        {
            "input": np.ascontiguousarray(x[c * BPC : (c + 1) * BPC]),
            "mask": np.ascontiguousarray(m[c * BPC : (c + 1) * BPC]),
        }
        for c in range(NCORES)
    ]


def _run(inputs, trace=False, **kw):
    from concourse.bass_utils import run_bass_kernel_spmd

    if trace:
        _enable_tracing()
    nc = _get_nc()
    res = run_bass_kernel_spmd(
        nc, _in_maps(inputs), core_ids=list(range(NCORES)), trace=trace, **kw
    )
    outs = np.stack([res.results[c]["out"] for c in range(NCORES)])  # [8, BPC, D]
    out_full = outs.reshape(B, 1, D).astype(np.float32)
    attn_mean = np.full((B, L), 1.0 / L, dtype=np.float32)
    return (out_full, attn_mean), res


def kernel(**inputs):
    (out_full, attn_mean), _ = _run(inputs, trace=False)
    return (out_full, attn_mean)


# revision 42
# speedup vs baseline: 1.2118x; 1.1815x over previous
"""Trainium2 Bass kernel for AttnSum3d pooling.

Math (per batch):
    xm = input * mask[:, None]                      # [L, D]
    S  = xm @ xm.T                                  # [L, L], symmetric
    w  = softmax(S, axis=0)  (columns sum to 1 over rows)
    out[d]       = (1/L) * sum_m sum_l w[l, m] xm[l, d]
    attn_mean[m] = (1/L) * sum_l w[l, m] = 1/L      (constant!)

Per m-block of 128 columns (stored row-wise thanks to symmetry):
    c[m]      >= max_l S[l, m]  via  sqrt(nsq[m]*max nsq) (Cauchy-Schwarz),
                 computed with a DVE exponent-halving sqrt (x1.06 margin)
    P_j[m, l] = exp(S_j[m, l] - c[m])     (ACT, bias=-c, accum_out=colsum)
    v[m]      = 1 / (L * colsum[m])
    r[l]     += sum_m v[m] * P_j[m, l]    (TensorE, lhsT=v, bf16)
    out[d]    = sum_l r[l] * xm[l, d]     (DVE accumulation + one
                                           partition-sum matmul)

Data-parallel over batch: 16 batches -> 8 cores x 2 batches. Batch 1's
prep and batch 0's tail are interleaved into the main loops to keep the
ACT engine (the bottleneck) fed continuously.
"""

import sys

for _p in ("/opt/trn_rl_repo",):
    if _p not in sys.path:
        sys.path.insert(0, _p)

import numpy as np

B, L, D = 16, 2048, 128
NCORES = 8
BPC = B // NCORES          # batches per core
NT = L // 128              # 16 tiles of 128 along L
TPB = L // 128             # L-rows per partition in the (p t) layout

_CACHE = {}


def _build_nc(batches=BPC):
    import concourse.bacc as bacc
    import concourse.tile as tile
    from concourse import mybir
    from concourse.masks import make_identity

    f32 = mybir.dt.float32
    bf16 = mybir.dt.bfloat16
    AF = mybir.ActivationFunctionType
    ALU = mybir.AluOpType
    AX = mybir.AxisListType

    nc = bacc.Bacc("TRN2", target_bir_lowering=False, debug=False)

    x_d = nc.dram_tensor("input", [BPC, L, D], f32, kind="ExternalInput").ap()
    m_d = nc.dram_tensor("mask", [BPC, L], f32, kind="ExternalInput").ap()
    o_d = nc.dram_tensor("out", [BPC, D], f32, kind="ExternalOutput").ap()

    with tile.TileContext(nc) as tc:
        with (
            tc.tile_pool(name="consts", bufs=1) as consts,
            tc.tile_pool(name="xb", bufs=2) as xb,
            tc.tile_pool(name="pb", bufs=6) as pb,
            tc.tile_pool(name="small", bufs=4) as small,
            tc.tile_pool(name="acc", bufs=2) as accp,
            tc.tile_pool(name="psS", bufs=2, space="PSUM") as psS,
            tc.tile_pool(name="psR", bufs=1, space="PSUM") as psR,
        ):
            identity = consts.tile([128, 128], f32)
            make_identity(nc, identity)
            ones_row = consts.tile([1, 128], f32)
            nc.vector.memset(ones_row, 1.0)
            ones_col = consts.tile([128, 1], f32)
            nc.vector.memset(ones_col, 1.0)

            ctxs = [{} for _ in range(batches)]
            # deferred closures, dependency-ordered and hand-interleaved so
            # PE-heavy ops (transposes) spread ~1-2 per block and never
            # crowd out S-matmul production in the static PE schedule
            deferred = []

            def drain(k):
                for _ in range(k):
                    if deferred:
                        deferred.pop(0)()

            def emit_loads(b):
                c = ctxs[b]
                c["xpa"] = xb.tile([128, 8, D], f32, name=f"xpa{b}", tag="xpa")
                c["xpb"] = xb.tile([128, 8, D], f32, name=f"xpb{b}", tag="xpb")
                xsrc = x_d[b].rearrange("(p t) d -> p t d", p=128)
                nc.sync.dma_start(out=c["xpa"][:], in_=xsrc[:, 0:8, :])
                nc.sync.dma_start(out=c["xpb"][:], in_=xsrc[:, 8:16, :])
                c["mask"] = xb.tile([128, TPB], f32, name=f"mask{b}", tag="mask")
                nc.sync.dma_start(
                    out=c["mask"][:], in_=m_d[b].rearrange("(p t) -> p t", p=128)
                )

            def xp_t(c, t):
                return c["xpa"][:, t, :] if t < 8 else c["xpb"][:, t - 8, :]

            def prep_ops(b):
                """Dependency-ordered closures for mask/nsq/negc/xmT with
                PE transposes interleaved between DVE ops."""
                c = ctxs[b]
                c["xm"] = xb.tile([128, NT, D], f32, name=f"xm{b}", tag="xm")
                c["nsq"] = xb.tile([128, NT], f32, name=f"nsq{b}", tag="nsq")
                c["xmT"] = xb.tile([128, L], bf16, name=f"xmT{b}", tag="xmT")
                masks = []
                nsqs = []
                trs = []
                finals = []

                for t in range(NT):
                    def _mask(t=t):
                        nc.vector.tensor_scalar_mul(
                            c["xm"][:, t, :], xp_t(c, t), c["mask"][:, t : t + 1]
                        )
                    masks.append(_mask)

                # nsq: batch 0 on the idle ACT (also pre-warms the exp
                # table); later batches on DVE (ACT is busy by then)
                for t in range(NT):
                    if b == 0:
                        def _nsq(t=t):
                            sqj = small.tile(
                                [128, D], f32, name=f"sqj{b}_{t}", tag="sqj"
                            )
                            nc.scalar.activation(
                                sqj[:],
                                xp_t(c, t),
                                AF.Square,
                                scale=c["mask"][:, t : t + 1],
                                accum_out=c["nsq"][:, t : t + 1],
                            )
                        nsqs.append(_nsq)
                    else:
                        def _nsq(t=t):
                            sqj = small.tile(
                                [128, D], f32, name=f"sqj{b}_{t}", tag="sqj"
                            )
                            nc.vector.tensor_mul(
                                sqj[:], c["xm"][:, t, :], c["xm"][:, t, :]
                            )
                            nc.vector.reduce_sum(
                                c["nsq"][:, t : t + 1], sqj[:], AX.X
                            )
                        nsqs.append(_nsq)

                def _n2max_a():
                    c["nmaxp"] = xb.tile([128, 1], f32, name=f"nmx{b}", tag="nmx")
                    nc.vector.reduce_max(c["nmaxp"][:], c["nsq"][:], AX.X)
                    tps = psS.tile([1, 128], f32, name=f"tpn{b}", tag="ps")
                    nc.tensor.transpose(tps[:], c["nmaxp"][:], identity[:])
                    c["nmr"] = xb.tile([1, 128], f32, name=f"nmr{b}", tag="nmr")
                    nc.vector.tensor_copy(c["nmr"][:], tps[:])
                finals.append(_n2max_a)

                def _n2max_b():
                    n2max = xb.tile([1, 1], f32, name=f"n2m{b}", tag="n2m")
                    nc.vector.reduce_max(n2max[:], c["nmr"][:], AX.X)
                    bps = psS.tile([128, 1], f32, name=f"bps{b}", tag="ps")
                    nc.tensor.matmul(
                        bps[:], ones_row[:], n2max[:], start=True, stop=True
                    )
                    c["n2b"] = xb.tile([128, 1], f32, name=f"n2b{b}", tag="n2b")
                    nc.vector.tensor_copy(c["n2b"][:], bps[:])
                finals.append(_n2max_b)

                def _negc():
                    zt = xb.tile([128, NT], f32, name=f"zt{b}", tag="zt")
                    nc.vector.tensor_scalar_mul(zt[:], c["nsq"][:], c["n2b"][:, 0:1])
                    zi = zt[:].bitcast(mybir.dt.int32)
                    nc.vector.tensor_scalar(zi, zi, 1, None, op0=ALU.arith_shift_right)
                    nc.vector.tensor_scalar(zi, zi, 0x1FC00000, None, op0=ALU.add)
                    c["negc"] = xb.tile([128, NT], f32, name=f"negc{b}", tag="negc")
                    nc.vector.tensor_scalar_mul(c["negc"][:], zt[:], -1.06)
                finals.append(_negc)

                for t in range(NT):
                    def _tr(t=t):
                        tp = psS.tile([128, 128], f32, name=f"tp{b}_{t}", tag="ps")
                        nc.tensor.transpose(tp[:], c["xm"][:, t, :], identity[:])
                        nc.vector.tensor_copy(
                            c["xmT"][:, t * 128 : (t + 1) * 128], tp[:]
                        )
                    trs.append(_tr)
                # interleave: all masks, then alternate nsq/transpose
                ops = list(masks)
                for t in range(NT):
                    ops.append(nsqs[t])
                    ops.append(trs[t])
                ops.extend(finals)
                return ops

            def emit_main(b):
                c = ctxs[b]
                c["r_ps"] = psR.tile([1, L], f32, name=f"r_ps{b}", tag="r")
                xmT = c["xmT"]
                pend = {}  # jb -> (vjb, Ph): vP runs one block behind so the
                # PE never waits on the exp/v-chain of the current block

                def emit_vP(jb):
                    vjb, Ph = pend.pop(jb)
                    for h in range(2):
                        for k in range(2):
                            nc.tensor.matmul(
                                c["r_ps"][
                                    0:1, h * 1024 + k * 512 : h * 1024 + (k + 1) * 512
                                ],
                                vjb[:],
                                Ph[h][:, k * 512 : (k + 1) * 512],
                                start=(jb == 0),
                                stop=(jb == NT - 1),
                            )

                for jb in range(NT):
                    lhsT = xmT[:, jb * 128 : (jb + 1) * 128]
                    csum = small.tile([128, 2], f32, name=f"cs{b}_{jb}", tag="cs")
                    Ph = []
                    for h in range(2):
                        S_ps = psS.tile(
                            [128, 1024], f32, name=f"S{b}_{jb}_{h}", tag="ps"
                        )
                        for k in range(2):
                            nc.tensor.matmul(
                                S_ps[:, k * 512 : (k + 1) * 512],
                                lhsT,
                                xmT[
                                    :, h * 1024 + k * 512 : h * 1024 + (k + 1) * 512
                                ],
                                start=True,
                                stop=True,
                            )
                        P = pb.tile([128, 1024], bf16, name=f"P{b}_{jb}_{h}", tag="P")
                        nc.scalar.activation(
                            P[:],
                            S_ps[:],
                            AF.Exp,
                            bias=c["negc"][:, jb : jb + 1],
                            scale=1.0,
                            accum_out=csum[:, h : h + 1],
                        )
                        Ph.append(P)

                    cst = small.tile([128, 1], f32, name=f"cst{b}_{jb}", tag="cst")
                    nc.vector.tensor_add(cst[:], csum[:, 0:1], csum[:, 1:2])
                    vj = small.tile([128, 1], f32, name=f"vj{b}_{jb}", tag="vj")
                    nc.vector.reciprocal(vj[:], cst[:])
                    vjb = small.tile([128, 1], bf16, name=f"vjb{b}_{jb}", tag="vjb")
                    nc.vector.tensor_scalar_mul(vjb[:], vj[:], 1.0 / L)
                    pend[jb] = (vjb, Ph)
                    if jb > 0:
                        emit_vP(jb - 1)
                    drain(4)
                emit_vP(NT - 1)

                c["r_sb"] = xb.tile([1, L], f32, name=f"r_sb{b}", tag="r_sb")

            def tail_ops(b):
                """r-psum drain + rT transposes + out accumulation, in
                dependency order with PE ops spread out."""
                c = ctxs[b]
                c["rT"] = xb.tile([128, NT], f32, name=f"rT{b}", tag="rT")
                cps = []
                for q in range(4):
                    def _cp(q=q):
                        nc.vector.tensor_copy(
                            c["r_sb"][0:1, q * 512 : (q + 1) * 512],
                            c["r_ps"][0:1, q * 512 : (q + 1) * 512],
                        )
                    cps.append(_cp)
                rts = []
                accs = []
                for i in range(NT):
                    def _rt(i=i):
                        tpr = psS.tile([128, 1], f32, name=f"tpr{b}_{i}", tag="ps")
                        nc.tensor.transpose(
                            tpr[:],
                            c["r_sb"][0:1, i * 128 : (i + 1) * 128],
                            identity[0:1, 0:1],
                        )
                        nc.vector.tensor_copy(c["rT"][:, i : i + 1], tpr[:])
                    rts.append(_rt)

                def _acc0():
                    a = accp.tile([128, D], f32, name=f"acc{b}_0", tag=f"acc{b}")
                    nc.vector.tensor_scalar_mul(
                        a[:], c["xm"][:, 0, :], c["rT"][:, 0:1]
                    )
                    c["acc"] = a
                accs.append(_acc0)
                for i in range(1, NT):
                    def _acci(i=i):
                        a = accp.tile([128, D], f32, name=f"acc{b}_{i}", tag=f"acc{b}")
                        nc.vector.scalar_tensor_tensor(
                            out=a[:],
                            in0=c["xm"][:, i, :],
                            scalar=c["rT"][:, i : i + 1],
                            in1=c["acc"][:],
                            op0=ALU.mult,
                            op1=ALU.add,
                        )
                        c["acc"] = a
                    accs.append(_acci)

                def _fin():
                    o_ps = psS.tile([1, D], f32, name=f"o_ps{b}", tag="ps")
                    nc.tensor.matmul(
                        o_ps[:], ones_col[:], c["acc"][:], start=True, stop=True
                    )
                    o_sb = xb.tile([1, D], f32, name=f"o_sb{b}", tag="o_sb")
                    nc.vector.tensor_copy(o_sb[:], o_ps[:])
                    nc.sync.dma_start(out=o_d[b : b + 1, :], in_=o_sb[:])
                ops = []
                for i in range(NT):
                    if i % 4 == 0:
                        ops.append(cps[i // 4])
                    ops.append(rts[i])
                    ops.append(accs[i])
                ops.append(_fin)
                return ops

            # ---------------- emission schedule ----------------
            for b in range(batches):
                emit_loads(b)
            for op in prep_ops(0):
                op()
            for b in range(batches):
                if b + 1 < batches:
                    deferred.extend(prep_ops(b + 1))
                emit_main(b)  # drains deferred (prep of b+1 / tail of b-1)
                deferred.extend(tail_ops(b))
            while deferred:
                deferred.pop(0)()

    nc.compile()
    return nc


def _get_nc():
    import os

    batches = int(os.environ.get("K_BATCHES", str(BPC)))
    key = ("nc", batches)
    if key not in _CACHE:
        _CACHE[key] = _build_nc(batches=batches)
    return _CACHE[key]


def _enable_tracing():
    """Shim antenv.axon_hooks (absent in this container) so
    run_bass_kernel_spmd(trace=True) can capture NTFF profiles through
    the axon .so, and neutralize the S3 artifact upload."""
    if _CACHE.get("trace_shim"):
        return
    import types

    import antenv

    if not hasattr(antenv, "axon_hooks"):
        mod = types.ModuleType("antenv.axon_hooks")
        mod._hook = None

        def set_axon_ntff_profile_hook(h):
            mod._hook = h

        def get_axon_ntff_profile_hook():
            return mod._hook

        mod.set_axon_ntff_profile_hook = set_axon_ntff_profile_hook
        mod.get_axon_ntff_profile_hook = get_axon_ntff_profile_hook
        sys.modules["antenv.axon_hooks"] = mod
        antenv.axon_hooks = mod

    from antenv.axon_hooks import get_axon_ntff_profile_hook, set_axon_ntff_profile_hook

    if get_axon_ntff_profile_hook() is None:
        if "/root/.axon_site" not in sys.path:
            sys.path.insert(0, "/root/.axon_site")
        from trn_agent_boot.trn_boot import _ntff_profile_via_ctypes

        set_axon_ntff_profile_hook(
            _ntff_profile_via_ctypes("/opt/axon/libaxon_pjrt.so")
        )

    import concourse.bass_utils as bu

    bu.upload_artifacts = lambda tmpdir: f"local://{tmpdir}"
    _CACHE["trace_shim"] = True


def _in_maps(inputs):
    x = np.ascontiguousarray(np.asarray(inputs["input"], dtype=np.float32))
    m = np.ascontiguousarray(np.asarray(inputs["mask"], dtype=np.float32))
    assert x.shape == (B, L, D) and m.shape == (B, L)
    return [
        {
            "input": np.ascontiguousarray(x[c * BPC : (c + 1) * BPC]),
            "mask": np.ascontiguousarray(m[c * BPC : (c + 1) * BPC]),
        }
        for c in range(NCORES)
    ]


def _run(inputs, trace=False, **kw):
    from concourse.bass_utils import run_bass_kernel_spmd

    if trace:
        _enable_tracing()
    nc = _get_nc()
    res = run_bass_kernel_spmd(
        nc, _in_maps(inputs), core_ids=list(range(NCORES)), trace=trace, **kw
    )
    outs = np.stack([res.results[c]["out"] for c in range(NCORES)])  # [8, BPC, D]
    out_full = outs.reshape(B, 1, D).astype(np.float32)
    attn_mean = np.full((B, L), 1.0 / L, dtype=np.float32)
    return (out_full, attn_mean), res


def kernel(**inputs):
    (out_full, attn_mean), _ = _run(inputs, trace=False)
    return (out_full, attn_mean)


# revision 43
# speedup vs baseline: 1.2859x; 1.0612x over previous
"""Trainium2 Bass kernel for AttnSum3d pooling.

Math (per batch):
    xm = input * mask[:, None]                      # [L, D]
    S  = xm @ xm.T                                  # [L, L], symmetric
    w  = softmax(S, axis=0)  (columns sum to 1 over rows)
    out[d]       = (1/L) * sum_m sum_l w[l, m] xm[l, d]
    attn_mean[m] = (1/L) * sum_l w[l, m] = 1/L      (constant!)

Per m-block of 128 columns (stored row-wise thanks to symmetry):
    c[m]      >= max_l S[l, m]  via  sqrt(nsq[m]*max nsq) (Cauchy-Schwarz),
                 computed with a DVE exponent-halving sqrt (x1.06 margin)
    P_j[m, l] = exp(S_j[m, l] - c[m])     (ACT, bias=-c, accum_out=colsum)
    v[m]      = 1 / (L * colsum[m])
    r[l]     += sum_m v[m] * P_j[m, l]    (TensorE, lhsT=v, bf16)
    out[d]    = sum_l r[l] * xm[l, d]     (DVE accumulation + one
                                           partition-sum matmul)

Data-parallel over batch: 16 batches -> 8 cores x 2 batches. Batch 1's
prep and batch 0's tail are interleaved into the main loops to keep the
ACT engine (the bottleneck) fed continuously.
"""

import sys

for _p in ("/opt/trn_rl_repo",):
    if _p not in sys.path:
        sys.path.insert(0, _p)

import numpy as np

B, L, D = 16, 2048, 128
NCORES = 8
BPC = B // NCORES          # batches per core
NT = L // 128              # 16 tiles of 128 along L
TPB = L // 128             # L-rows per partition in the (p t) layout

_CACHE = {}


def _build_nc(batches=BPC):
    import concourse.bacc as bacc
    import concourse.tile as tile
    from concourse import mybir
    from concourse.masks import make_identity

    f32 = mybir.dt.float32
    bf16 = mybir.dt.bfloat16
    AF = mybir.ActivationFunctionType
    ALU = mybir.AluOpType
    AX = mybir.AxisListType

    nc = bacc.Bacc("TRN2", target_bir_lowering=False, debug=False)

    x_d = nc.dram_tensor("input", [BPC, L, D], f32, kind="ExternalInput").ap()
    m_d = nc.dram_tensor("mask", [BPC, L], f32, kind="ExternalInput").ap()
    o_d = nc.dram_tensor("out", [BPC, D], f32, kind="ExternalOutput").ap()

    with tile.TileContext(nc) as tc:
        with (
            tc.tile_pool(name="consts", bufs=1) as consts,
            tc.tile_pool(name="xb", bufs=2) as xb,
            tc.tile_pool(name="pb", bufs=6) as pb,
            tc.tile_pool(name="small", bufs=4) as small,
            tc.tile_pool(name="acc", bufs=2) as accp,
            tc.tile_pool(name="psS", bufs=2, space="PSUM") as psS,
            tc.tile_pool(name="psR", bufs=1, space="PSUM") as psR,
        ):
            identity = consts.tile([128, 128], f32)
            make_identity(nc, identity)
            ones_row = consts.tile([1, 128], f32)
            nc.vector.memset(ones_row, 1.0)
            ones_col = consts.tile([128, 1], f32)
            nc.vector.memset(ones_col, 1.0)

            ctxs = [{} for _ in range(batches)]
            # deferred closures, dependency-ordered and hand-interleaved so
            # PE-heavy ops (transposes) spread ~1-2 per block and never
            # crowd out S-matmul production in the static PE schedule
            deferred = []

            def drain(k):
                for _ in range(k):
                    if deferred:
                        deferred.pop(0)()

            def emit_loads(b):
                c = ctxs[b]
                c["xpa"] = xb.tile([128, 8, D], f32, name=f"xpa{b}", tag="xpa")
                c["xpb"] = xb.tile([128, 8, D], f32, name=f"xpb{b}", tag="xpb")
                xsrc = x_d[b].rearrange("(p t) d -> p t d", p=128)
                nc.sync.dma_start(out=c["xpa"][:], in_=xsrc[:, 0:8, :])
                nc.sync.dma_start(out=c["xpb"][:], in_=xsrc[:, 8:16, :])
                c["mask"] = xb.tile([128, TPB], f32, name=f"mask{b}", tag="mask")
                nc.sync.dma_start(
                    out=c["mask"][:], in_=m_d[b].rearrange("(p t) -> p t", p=128)
                )

            def xp_t(c, t):
                return c["xpa"][:, t, :] if t < 8 else c["xpb"][:, t - 8, :]

            def prep_ops(b):
                """Dependency-ordered closures for mask/nsq/negc/xmT with
                PE transposes interleaved between DVE ops."""
                c = ctxs[b]
                c["xm"] = xb.tile([128, NT, D], f32, name=f"xm{b}", tag="xm")
                c["nsq"] = xb.tile([128, NT], f32, name=f"nsq{b}", tag="nsq")
                c["xmT"] = xb.tile([128, L], bf16, name=f"xmT{b}", tag="xmT")
                masks = []
                nsqs = []
                trs = []
                finals = []

                for t in range(NT):
                    def _mask(t=t):
                        nc.vector.tensor_scalar_mul(
                            c["xm"][:, t, :], xp_t(c, t), c["mask"][:, t : t + 1]
                        )
                    masks.append(_mask)

                # nsq: batch 0 on the idle ACT (also pre-warms the exp
                # table); later batches on DVE (ACT is busy by then)
                for t in range(NT):
                    if b == 0:
                        def _nsq(t=t):
                            sqj = small.tile(
                                [128, D], f32, name=f"sqj{b}_{t}", tag="sqj"
                            )
                            nc.scalar.activation(
                                sqj[:],
                                xp_t(c, t),
                                AF.Square,
                                scale=c["mask"][:, t : t + 1],
                                accum_out=c["nsq"][:, t : t + 1],
                            )
                        nsqs.append(_nsq)
                    else:
                        def _nsq(t=t):
                            sqj = small.tile(
                                [128, D], f32, name=f"sqj{b}_{t}", tag="sqj"
                            )
                            nc.vector.tensor_mul(
                                sqj[:], c["xm"][:, t, :], c["xm"][:, t, :]
                            )
                            nc.vector.reduce_sum(
                                c["nsq"][:, t : t + 1], sqj[:], AX.X
                            )
                        nsqs.append(_nsq)

                def _n2max_a():
                    c["nmaxp"] = xb.tile([128, 1], f32, name=f"nmx{b}", tag="nmx")
                    nc.vector.reduce_max(c["nmaxp"][:], c["nsq"][:], AX.X)
                    tps = psS.tile([1, 128], f32, name=f"tpn{b}", tag="ps")
                    nc.tensor.transpose(tps[:], c["nmaxp"][:], identity[:])
                    c["nmr"] = xb.tile([1, 128], f32, name=f"nmr{b}", tag="nmr")
                    nc.vector.tensor_copy(c["nmr"][:], tps[:])
                finals.append(_n2max_a)

                def _n2max_b():
                    n2max = xb.tile([1, 1], f32, name=f"n2m{b}", tag="n2m")
                    nc.vector.reduce_max(n2max[:], c["nmr"][:], AX.X)
                    bps = psS.tile([128, 1], f32, name=f"bps{b}", tag="ps")
                    nc.tensor.matmul(
                        bps[:], ones_row[:], n2max[:], start=True, stop=True
                    )
                    c["n2b"] = xb.tile([128, 1], f32, name=f"n2b{b}", tag="n2b")
                    nc.vector.tensor_copy(c["n2b"][:], bps[:])
                finals.append(_n2max_b)

                def _negc():
                    zt = xb.tile([128, NT], f32, name=f"zt{b}", tag="zt")
                    nc.vector.tensor_scalar_mul(zt[:], c["nsq"][:], c["n2b"][:, 0:1])
                    zi = zt[:].bitcast(mybir.dt.int32)
                    nc.vector.tensor_scalar(zi, zi, 1, None, op0=ALU.arith_shift_right)
                    nc.vector.tensor_scalar(zi, zi, 0x1FC00000, None, op0=ALU.add)
                    c["negc"] = xb.tile([128, NT], f32, name=f"negc{b}", tag="negc")
                    nc.vector.tensor_scalar_mul(c["negc"][:], zt[:], -1.06)
                finals.append(_negc)

                if b == 0:
                    # preamble: PE is idle, use TensorE transposes
                    for t in range(NT):
                        def _tr(t=t):
                            tp = psS.tile(
                                [128, 128], f32, name=f"tp{b}_{t}", tag="ps"
                            )
                            nc.tensor.transpose(tp[:], c["xm"][:, t, :], identity[:])
                            nc.vector.tensor_copy(
                                c["xmT"][:, t * 128 : (t + 1) * 128], tp[:]
                            )
                        trs.append(_tr)
                else:
                    # mid-run: PE has no slack (it delays S production and
                    # starves ACT) -> use the idle DMA xbar instead
                    c["xm_bf"] = xb.tile(
                        [128, NT, D], bf16, name=f"xmbf{b}", tag="xmbf"
                    )
                    def _cast():
                        nc.vector.tensor_copy(c["xm_bf"][:], c["xm"][:])
                    trs.append(_cast)
                    for t in range(NT):
                        def _tr(t=t):
                            nc.sync.dma_start_transpose(
                                out=c["xmT"][:, t * 128 : (t + 1) * 128],
                                in_=c["xm_bf"][:, t, :],
                            )
                        trs.append(_tr)
                # interleave: all masks, then nsq/transpose alternating
                ops = list(masks)
                n0, n1 = len(nsqs), len(trs)
                i = j = 0
                while i < n0 or j < n1:
                    if i < n0:
                        ops.append(nsqs[i]); i += 1
                    if j < n1:
                        ops.append(trs[j]); j += 1
                ops.extend(finals)
                return ops

            def emit_main(b):
                c = ctxs[b]
                c["r_ps"] = psR.tile([1, L], f32, name=f"r_ps{b}", tag="r")
                xmT = c["xmT"]
                pend = {}  # jb -> (vjb, Ph): vP runs one block behind so the
                # PE never waits on the exp/v-chain of the current block

                def emit_vP(jb):
                    vjb, Ph = pend.pop(jb)
                    for h in range(2):
                        for k in range(2):
                            nc.tensor.matmul(
                                c["r_ps"][
                                    0:1, h * 1024 + k * 512 : h * 1024 + (k + 1) * 512
                                ],
                                vjb[:],
                                Ph[h][:, k * 512 : (k + 1) * 512],
                                start=(jb == 0),
                                stop=(jb == NT - 1),
                            )

                for jb in range(NT):
                    lhsT = xmT[:, jb * 128 : (jb + 1) * 128]
                    csum = small.tile([128, 2], f32, name=f"cs{b}_{jb}", tag="cs")
                    Ph = []
                    for h in range(2):
                        S_ps = psS.tile(
                            [128, 1024], f32, name=f"S{b}_{jb}_{h}", tag="ps"
                        )
                        for k in range(2):
                            nc.tensor.matmul(
                                S_ps[:, k * 512 : (k + 1) * 512],
                                lhsT,
                                xmT[
                                    :, h * 1024 + k * 512 : h * 1024 + (k + 1) * 512
                                ],
                                start=True,
                                stop=True,
                            )
                        P = pb.tile([128, 1024], bf16, name=f"P{b}_{jb}_{h}", tag="P")
                        nc.scalar.activation(
                            P[:],
                            S_ps[:],
                            AF.Exp,
                            bias=c["negc"][:, jb : jb + 1],
                            scale=1.0,
                            accum_out=csum[:, h : h + 1],
                        )
                        Ph.append(P)

                    cst = small.tile([128, 1], f32, name=f"cst{b}_{jb}", tag="cst")
                    nc.vector.tensor_add(cst[:], csum[:, 0:1], csum[:, 1:2])
                    vj = small.tile([128, 1], f32, name=f"vj{b}_{jb}", tag="vj")
                    nc.vector.reciprocal(vj[:], cst[:])
                    vjb = small.tile([128, 1], bf16, name=f"vjb{b}_{jb}", tag="vjb")
                    nc.vector.tensor_scalar_mul(vjb[:], vj[:], 1.0 / L)
                    pend[jb] = (vjb, Ph)
                    if jb > 0:
                        emit_vP(jb - 1)
                    drain(4)
                emit_vP(NT - 1)

                c["r_sb"] = xb.tile([1, L], f32, name=f"r_sb{b}", tag="r_sb")

            def tail_ops(b):
                """r-psum drain + rT transposes + out accumulation, in
                dependency order with PE ops spread out."""
                c = ctxs[b]
                c["rT"] = xb.tile([128, NT], f32, name=f"rT{b}", tag="rT")
                cps = []
                for q in range(4):
                    def _cp(q=q):
                        nc.vector.tensor_copy(
                            c["r_sb"][0:1, q * 512 : (q + 1) * 512],
                            c["r_ps"][0:1, q * 512 : (q + 1) * 512],
                        )
                    cps.append(_cp)
                rts = []
                accs = []
                for i in range(NT):
                    def _rt(i=i):
                        tpr = psS.tile([128, 1], f32, name=f"tpr{b}_{i}", tag="ps")
                        nc.tensor.transpose(
                            tpr[:],
                            c["r_sb"][0:1, i * 128 : (i + 1) * 128],
                            identity[0:1, 0:1],
                        )
                        nc.vector.tensor_copy(c["rT"][:, i : i + 1], tpr[:])
                    rts.append(_rt)

                def _acc0():
                    a = accp.tile([128, D], f32, name=f"acc{b}_0", tag=f"acc{b}")
                    nc.vector.tensor_scalar_mul(
                        a[:], c["xm"][:, 0, :], c["rT"][:, 0:1]
                    )
                    c["acc"] = a
                accs.append(_acc0)
                for i in range(1, NT):
                    def _acci(i=i):
                        a = accp.tile([128, D], f32, name=f"acc{b}_{i}", tag=f"acc{b}")
                        nc.vector.scalar_tensor_tensor(
                            out=a[:],
                            in0=c["xm"][:, i, :],
                            scalar=c["rT"][:, i : i + 1],
                            in1=c["acc"][:],
                            op0=ALU.mult,
                            op1=ALU.add,
                        )
                        c["acc"] = a
                    accs.append(_acci)

                def _fin():
                    o_ps = psS.tile([1, D], f32, name=f"o_ps{b}", tag="ps")
                    nc.tensor.matmul(
                        o_ps[:], ones_col[:], c["acc"][:], start=True, stop=True
                    )
                    o_sb = xb.tile([1, D], f32, name=f"o_sb{b}", tag="o_sb")
                    nc.vector.tensor_copy(o_sb[:], o_ps[:])
                    nc.sync.dma_start(out=o_d[b : b + 1, :], in_=o_sb[:])
                ops = []
                for i in range(NT):
                    if i % 4 == 0:
                        ops.append(cps[i // 4])
                    ops.append(rts[i])
                    ops.append(accs[i])
                ops.append(_fin)
                return ops

            # ---------------- emission schedule ----------------
            for b in range(batches):
                emit_loads(b)
            for op in prep_ops(0):
                op()
            for b in range(batches):
                if b + 1 < batches:
                    deferred.extend(prep_ops(b + 1))
                emit_main(b)  # drains deferred (prep of b+1 / tail of b-1)
                deferred.extend(tail_ops(b))
            while deferred:
                deferred.pop(0)()

    nc.compile()
    return nc


def _get_nc():
    import os

    batches = int(os.environ.get("K_BATCHES", str(BPC)))
    key = ("nc", batches)
    if key not in _CACHE:
        _CACHE[key] = _build_nc(batches=batches)
    return _CACHE[key]


def _enable_tracing():
    """Shim antenv.axon_hooks (absent in this container) so
    run_bass_kernel_spmd(trace=True) can capture NTFF profiles through
    the axon .so, and neutralize the S3 artifact upload."""
    if _CACHE.get("trace_shim"):
        return
    import types

    import antenv

    if not hasattr(antenv, "axon_hooks"):
        mod = types.ModuleType("antenv.axon_hooks")
        mod._hook = None

        def set_axon_ntff_profile_hook(h):
            mod._hook = h

        def get_axon_ntff_profile_hook():
            return mod._hook

        mod.set_axon_ntff_profile_hook = set_axon_ntff_profile_hook
        mod.get_axon_ntff_profile_hook = get_axon_ntff_profile_hook
        sys.modules["antenv.axon_hooks"] = mod
        antenv.axon_hooks = mod

    from antenv.axon_hooks import get_axon_ntff_profile_hook, set_axon_ntff_profile_hook

    if get_axon_ntff_profile_hook() is None:
        if "/root/.axon_site" not in sys.path:
            sys.path.insert(0, "/root/.axon_site")
        from trn_agent_boot.trn_boot import _ntff_profile_via_ctypes

        set_axon_ntff_profile_hook(
            _ntff_profile_via_ctypes("/opt/axon/libaxon_pjrt.so")
        )

    import concourse.bass_utils as bu

    bu.upload_artifacts = lambda tmpdir: f"local://{tmpdir}"
    _CACHE["trace_shim"] = True


def _in_maps(inputs):
    x = np.ascontiguousarray(np.asarray(inputs["input"], dtype=np.float32))
    m = np.ascontiguousarray(np.asarray(inputs["mask"], dtype=np.float32))
    assert x.shape == (B, L, D) and m.shape == (B, L)
    return [
        {
            "input": np.ascontiguousarray(x[c * BPC : (c + 1) * BPC]),
            "mask": np.ascontiguousarray(m[c * BPC : (c + 1) * BPC]),
        }
        for c in range(NCORES)
    ]


def _run(inputs, trace=False, **kw):
    from concourse.bass_utils import run_bass_kernel_spmd

    if trace:
        _enable_tracing()
    nc = _get_nc()
    res = run_bass_kernel_spmd(
        nc, _in_maps(inputs), core_ids=list(range(NCORES)), trace=trace, **kw
    )
    outs = np.stack([res.results[c]["out"] for c in range(NCORES)])  # [8, BPC, D]
    out_full = outs.reshape(B, 1, D).astype(np.float32)
    attn_mean = np.full((B, L), 1.0 / L, dtype=np.float32)
    return (out_full, attn_mean), res


def kernel(**inputs):
    (out_full, attn_mean), _ = _run(inputs, trace=False)
    return (out_full, attn_mean)
